# revision 9
# baseline (speedup 1.0000x reference)
"""Locally-connected graph-conv kernel for Trainium2 (Bass/Tile).

Computes out[b,t,m] = sum_n x[b,t,n] * (S*W)[n,m] + bias[m] for
x [64, 2048, 208], W/S [208, 208], bias [208].

The ring-graph support S is a +-4 band (mod 208), so each half of the
output nodes only needs a 113-row slice of the contraction dim:
  block 0 (m 0..103):   n in {204..207} ++ {0..107}  + ones row (bias)
  block 1 (m 104..207): n in {100..207} ++ {0..3}    + ones row (bias)
That makes each output block a SINGLE [113,104] x [113,512] matmul with
the masked-weight block stationary in the PE array and x^T streaming as
the moving operand in 512-column blocks (long streams hide the fp32
LDWEIGHTS).

Data-parallel over 8 NeuronCores: each core gets 16384 rows of the
flattened [131072, 208] x. The host pre-assembles the two halo row-blocks
(transposed, ones row appended) into one [226, 16384] tensor so each
moving tile is a single contiguous-row DMA. Loads issue on the Sync HWDGE
ring, stores on the Scalar ring. The host transposes y^T back at gather.
"""

import numpy as np
from contextlib import ExitStack

import concourse.bacc as bacc
import concourse.mybir as mybir
import concourse.tile as tile
from concourse.bass_utils import run_bass_kernel_spmd

N = 208                      # nodes
HALF = 104                   # output nodes per block
K = 4                        # band half-width of S
NH = 2 * K + HALF            # 112 contraction rows per block (halo incl.)
NHE = NH + 1                 # 113 (+ ones row carrying the bias)
N_CORES = 8
B, T = 64, 2048
ROWS_TOTAL = B * T           # 131072
SHARD = ROWS_TOTAL // N_CORES    # 16384 rows per core
TB = 512                     # moving-block columns per matmul (fp32 PSUM max)
TOUT = 2048                  # t-columns per DMA chunk (~0.9 MB loads)
N_CHUNKS = SHARD // TOUT     # 8
SUB = TOUT // TB             # 4 matmul sub-blocks per chunk

FP32 = mybir.dt.float32

# halo row order (indices into the [208] node dim) for each block
ROWS0 = list(range(N - K, N)) + list(range(0, HALF + K))          # 112
ROWS1 = list(range(HALF - K, N)) + list(range(0, K))              # 112

_CACHE = {}
LAST_RESULTS = None          # BassKernelResults of the most recent run


def _kernel_body(tc):
    nc = tc.nc
    # pre-assembled halo blocks: rows 0:113 block0, 113:226 block1
    x_d = nc.dram_tensor("xh", [2 * NHE, SHARD], FP32, kind="ExternalInput").ap()
    w_d = nc.dram_tensor("w", [N, N], FP32, kind="ExternalInput").ap()
    s_d = nc.dram_tensor("s", [N, N], FP32, kind="ExternalInput").ap()
    b_d = nc.dram_tensor("bias", [1, N], FP32, kind="ExternalInput").ap()
    o_d = nc.dram_tensor("outt", [N, SHARD], FP32, kind="ExternalOutput").ap()

    with ExitStack() as ctx:
        const = ctx.enter_context(tc.tile_pool(name="const", bufs=1))

        # Stationary blocks wh0/wh1 [113, 104]: masked weight rows in halo
        # order + bias as the last contraction row.
        w0 = const.tile([NH, HALF], FP32, tag="w0")
        s0 = const.tile([NH, HALF], FP32, tag="s0")
        nc.sync.dma_start(w0[0:K, :], w_d[N - K : N, 0:HALF])
        nc.sync.dma_start(w0[K:NH, :], w_d[0 : HALF + K, 0:HALF])
        nc.sync.dma_start(s0[0:K, :], s_d[N - K : N, 0:HALF])
        nc.sync.dma_start(s0[K:NH, :], s_d[0 : HALF + K, 0:HALF])
        wh0 = const.tile([NHE, HALF], FP32, tag="wh0")
        nc.vector.tensor_mul(wh0[0:NH, :], w0, s0)
        nc.sync.dma_start(wh0[NH:NHE, :], b_d[:, 0:HALF])
        w1 = const.tile([NH, HALF], FP32, tag="w1")
        s1 = const.tile([NH, HALF], FP32, tag="s1")
        nc.sync.dma_start(w1[0 : HALF + K, :], w_d[HALF - K : N, HALF:N])
        nc.sync.dma_start(w1[HALF + K : NH, :], w_d[0:K, HALF:N])
        nc.sync.dma_start(s1[0 : HALF + K, :], s_d[HALF - K : N, HALF:N])
        nc.sync.dma_start(s1[HALF + K : NH, :], s_d[0:K, HALF:N])
        wh1 = const.tile([NHE, HALF], FP32, tag="wh1")
        nc.vector.tensor_mul(wh1[0:NH, :], w1, s1)
        nc.sync.dma_start(wh1[NH:NHE, :], b_d[:, HALF:N])

        x0p = ctx.enter_context(tc.tile_pool(name="x0p", bufs=3))
        x1p = ctx.enter_context(tc.tile_pool(name="x1p", bufs=3))
        o0p = ctx.enter_context(tc.tile_pool(name="o0p", bufs=3))
        o1p = ctx.enter_context(tc.tile_pool(name="o1p", bufs=3))
        ps0p = ctx.enter_context(tc.tile_pool(name="ps0p", bufs=4, space="PSUM"))
        ps1p = ctx.enter_context(tc.tile_pool(name="ps1p", bufs=4, space="PSUM"))

        for c in range(N_CHUNKS):
            tsl = slice(c * TOUT, (c + 1) * TOUT)
            xh0 = x0p.tile([NHE, TOUT], FP32, tag="xh0")
            nc.sync.dma_start(xh0, x_d[0:NHE, tsl])
            xh1 = x1p.tile([NHE, TOUT], FP32, tag="xh1")
            nc.sync.dma_start(xh1, x_d[NHE : 2 * NHE, tsl])

            o0_t = o0p.tile([HALF, TOUT], FP32, tag="o0")
            o1_t = o1p.tile([HALF, TOUT], FP32, tag="o1")
            for s in range(SUB):
                sl = slice(s * TB, (s + 1) * TB)
                ps0 = ps0p.tile([HALF, TB], FP32, tag="ps0")
                nc.tensor.matmul(ps0, wh0, xh0[:, sl], start=True, stop=True)
                ps1 = ps1p.tile([HALF, TB], FP32, tag="ps1")
                nc.tensor.matmul(ps1, wh1, xh1[:, sl], start=True, stop=True)
                nc.scalar.copy(o0_t[:, sl], ps0)
                nc.vector.tensor_copy(o1_t[:, sl], ps1)
            # stores go out on the Scalar-engine HWDGE ring
            nc.scalar.dma_start(o_d[0:HALF, tsl], o0_t)
            nc.scalar.dma_start(o_d[HALF:N, tsl], o1_t)


def _build():
    nc = bacc.Bacc(
        "TRN2",
        target_bir_lowering=False,
        debug=False,
        num_devices=N_CORES,
    )
    with tile.TileContext(nc) as tc:
        _kernel_body(tc)
    nc.compile()
    return nc


def kernel(x, W, b, S):
    global LAST_RESULTS
    nc = _CACHE.get("nc")
    if nc is None:
        nc = _build()
        _CACHE["nc"] = nc

    xf = np.asarray(x, np.float32).reshape(ROWS_TOTAL, N)
    Wf = np.ascontiguousarray(np.asarray(W, np.float32))
    Sf = np.ascontiguousarray(np.asarray(S, np.float32))
    bf = np.ascontiguousarray(np.asarray(b, np.float32).reshape(1, N))

    in_maps = []
    for i in range(N_CORES):
        xt = xf[i * SHARD : (i + 1) * SHARD].T          # [208, SHARD] view
        xh = np.empty((2 * NHE, SHARD), np.float32)
        xh[0:NH] = xt[ROWS0]
        xh[NH] = 1.0
        xh[NHE : NHE + NH] = xt[ROWS1]
        xh[NHE + NH] = 1.0
        in_maps.append({"xh": xh, "w": Wf, "s": Sf, "bias": bf})
    res = run_bass_kernel_spmd(nc, in_maps, core_ids=list(range(N_CORES)))
    LAST_RESULTS = res
    out = np.empty((ROWS_TOTAL, N), np.float32)
    for i, r in enumerate(res.results):
        out[i * SHARD : (i + 1) * SHARD] = r["outt"].T
    return out.reshape(B, T, N)


# revision 10
# speedup vs baseline: 5.4567x; 5.4567x over previous
"""Locally-connected graph-conv kernel for Trainium2 (Bass/Tile).

Computes out[b,t,m] = sum_n x[b,t,n] * (S*W)[n,m] + bias[m] for
x [64, 2048, 208], W/S [208, 208], bias [208].

The ring-graph support S is a +-4 band (mod 208), so each half of the
output nodes only needs a 112-row slice of the contraction dim:
  block 0 (m 0..103):   n in {204..207} ++ {0..107}   (+ ones row = bias)
  block 1 (m 104..207): n in {100..207} ++ {0..3}     (+ ones row = bias)
Each output block is then a SINGLE [113,104] x [113,512] matmul with the
masked-weight block stationary in the PE array and x^T streaming as the
moving operand in 512-column blocks (long streams hide fp32 LDWEIGHTS).

Data-parallel over 8 NeuronCores: each core gets 16384 rows of the
flattened x, host-pre-assembled into a [225, 16384] tensor (two 112-row
halo blocks + ones row). DMA partition counts are kept multiples of 16
(the fast HWDGE path: ~250 GB/s/instr vs ~27 otherwise): halo loads are
[112]-row DMAs + a 1-row ones DMA; stores are [112]-row DMAs into a
[224, SHARD] output (8 pad rows per block, dropped at host gather).
Loads issue on the Sync ring, stores on the Scalar ring. The host
transposes y^T back at gather.
"""

import numpy as np
from contextlib import ExitStack

import concourse.bacc as bacc
import concourse.mybir as mybir
import concourse.tile as tile
from concourse.bass_utils import run_bass_kernel_spmd

N = 208                      # nodes
HALF = 104                   # output nodes per block
K = 4                        # band half-width of S
NH = 2 * K + HALF            # 112 contraction rows per block (halo incl.)
NHE = NH + 1                 # 113 (+ ones row carrying the bias)
NP = 112                     # padded store rows (multiple of 16)
N_CORES = 8
B, T = 64, 2048
ROWS_TOTAL = B * T           # 131072
SHARD = ROWS_TOTAL // N_CORES    # 16384 rows per core
TB = 512                     # moving-block columns per matmul (fp32 PSUM max)
TOUT = 2048                  # t-columns per DMA chunk (~0.9 MB loads)
N_CHUNKS = SHARD // TOUT     # 8
SUB = TOUT // TB             # 4 matmul sub-blocks per chunk

FP32 = mybir.dt.float32

# halo row order (indices into the [208] node dim) for each block
ROWS0 = list(range(N - K, N)) + list(range(0, HALF + K))          # 112
ROWS1 = list(range(HALF - K, N)) + list(range(0, K))              # 112

_CACHE = {}
LAST_RESULTS = None          # BassKernelResults of the most recent run


def _kernel_body(tc):
    nc = tc.nc
    # rows 0:112 block0 halo, 112:224 block1 halo, 224 ones
    x_d = nc.dram_tensor("xh", [2 * NH + 1, SHARD], FP32, kind="ExternalInput").ap()
    w_d = nc.dram_tensor("w", [N, N], FP32, kind="ExternalInput").ap()
    s_d = nc.dram_tensor("s", [N, N], FP32, kind="ExternalInput").ap()
    b_d = nc.dram_tensor("bias", [1, N], FP32, kind="ExternalInput").ap()
    o_d = nc.dram_tensor("outt", [2 * NP, SHARD], FP32, kind="ExternalOutput").ap()

    with ExitStack() as ctx:
        const = ctx.enter_context(tc.tile_pool(name="const", bufs=1))

        # Stationary blocks wh0/wh1 [113, 104]: masked weight rows in halo
        # order + bias as the last contraction row.
        w0 = const.tile([NH, HALF], FP32, tag="w0")
        s0 = const.tile([NH, HALF], FP32, tag="s0")
        nc.sync.dma_start(w0[0:K, :], w_d[N - K : N, 0:HALF])
        nc.sync.dma_start(w0[K:NH, :], w_d[0 : HALF + K, 0:HALF])
        nc.sync.dma_start(s0[0:K, :], s_d[N - K : N, 0:HALF])
        nc.sync.dma_start(s0[K:NH, :], s_d[0 : HALF + K, 0:HALF])
        wh0 = const.tile([NHE, HALF], FP32, tag="wh0")
        nc.vector.tensor_mul(wh0[0:NH, :], w0, s0)
        nc.sync.dma_start(wh0[NH:NHE, :], b_d[:, 0:HALF])
        w1 = const.tile([NH, HALF], FP32, tag="w1")
        s1 = const.tile([NH, HALF], FP32, tag="s1")
        nc.sync.dma_start(w1[0 : HALF + K, :], w_d[HALF - K : N, HALF:N])
        nc.sync.dma_start(w1[HALF + K : NH, :], w_d[0:K, HALF:N])
        nc.sync.dma_start(s1[0 : HALF + K, :], s_d[HALF - K : N, HALF:N])
        nc.sync.dma_start(s1[HALF + K : NH, :], s_d[0:K, HALF:N])
        wh1 = const.tile([NHE, HALF], FP32, tag="wh1")
        nc.vector.tensor_mul(wh1[0:NH, :], w1, s1)
        nc.sync.dma_start(wh1[NH:NHE, :], b_d[:, HALF:N])

        x0p = ctx.enter_context(tc.tile_pool(name="x0p", bufs=3))
        x1p = ctx.enter_context(tc.tile_pool(name="x1p", bufs=3))
        o0p = ctx.enter_context(tc.tile_pool(name="o0p", bufs=3))
        o1p = ctx.enter_context(tc.tile_pool(name="o1p", bufs=3))
        ps0p = ctx.enter_context(tc.tile_pool(name="ps0p", bufs=4, space="PSUM"))
        ps1p = ctx.enter_context(tc.tile_pool(name="ps1p", bufs=4, space="PSUM"))

        for c in range(N_CHUNKS):
            tsl = slice(c * TOUT, (c + 1) * TOUT)
            xh0 = x0p.tile([NHE, TOUT], FP32, tag="xh0")
            nc.sync.dma_start(xh0[0:NH, :], x_d[0:NH, tsl])
            nc.sync.dma_start(xh0[NH:NHE, :], x_d[2 * NH : 2 * NH + 1, tsl])
            xh1 = x1p.tile([NHE, TOUT], FP32, tag="xh1")
            nc.sync.dma_start(xh1[0:NH, :], x_d[NH : 2 * NH, tsl])
            nc.sync.dma_start(xh1[NH:NHE, :], x_d[2 * NH : 2 * NH + 1, tsl])

            o0_t = o0p.tile([NP, TOUT], FP32, tag="o0")
            o1_t = o1p.tile([NP, TOUT], FP32, tag="o1")
            for s in range(SUB):
                sl = slice(s * TB, (s + 1) * TB)
                ps0 = ps0p.tile([HALF, TB], FP32, tag="ps0")
                nc.tensor.matmul(ps0, wh0, xh0[:, sl], start=True, stop=True)
                ps1 = ps1p.tile([HALF, TB], FP32, tag="ps1")
                nc.tensor.matmul(ps1, wh1, xh1[:, sl], start=True, stop=True)
                nc.scalar.copy(o0_t[0:HALF, sl], ps0)
                nc.vector.tensor_copy(o1_t[0:HALF, sl], ps1)
            # stores (112 rows, 8 pad) on the Scalar-engine HWDGE ring
            nc.scalar.dma_start(o_d[0:NP, tsl], o0_t)
            nc.scalar.dma_start(o_d[NP : 2 * NP, tsl], o1_t)


def _build():
    nc = bacc.Bacc(
        "TRN2",
        target_bir_lowering=False,
        debug=False,
        num_devices=N_CORES,
    )
    with tile.TileContext(nc) as tc:
        _kernel_body(tc)
    nc.compile()
    return nc


def kernel(x, W, b, S):
    global LAST_RESULTS
    nc = _CACHE.get("nc")
    if nc is None:
        nc = _build()
        _CACHE["nc"] = nc

    xf = np.asarray(x, np.float32).reshape(ROWS_TOTAL, N)
    Wf = np.ascontiguousarray(np.asarray(W, np.float32))
    Sf = np.ascontiguousarray(np.asarray(S, np.float32))
    bf = np.ascontiguousarray(np.asarray(b, np.float32).reshape(1, N))

    in_maps = []
    for i in range(N_CORES):
        xt = xf[i * SHARD : (i + 1) * SHARD].T          # [208, SHARD] view
        xh = np.empty((2 * NH + 1, SHARD), np.float32)
        xh[0:NH] = xt[ROWS0]
        xh[NH : 2 * NH] = xt[ROWS1]
        xh[2 * NH] = 1.0
        in_maps.append({"xh": xh, "w": Wf, "s": Sf, "bias": bf})
    res = run_bass_kernel_spmd(nc, in_maps, core_ids=list(range(N_CORES)))
    LAST_RESULTS = res
    out = np.empty((ROWS_TOTAL, N), np.float32)
    for i, r in enumerate(res.results):
        yt = r["outt"]                                  # [224, SHARD]
        out[i * SHARD : (i + 1) * SHARD, 0:HALF] = yt[0:HALF].T
        out[i * SHARD : (i + 1) * SHARD, HALF:N] = yt[NP : NP + HALF].T
    return out.reshape(B, T, N)
